# revision 1
# baseline (speedup 1.0000x reference)
"""Trainium2 Bass kernel for an 8-step complex DMD recurrence.

Math (matching the reference):
  Ag[0]=A[0], Ag[p]=A[8-p] (p>=1), all complex [M,M].
  uc window w_t (len 8) starts as the real inputs x_0..x_7; each step
    u2_t = sum_p Ag[p] @ w_t[p]   (complex, [B,M])
  then the window slides.  Output = Re([u2_1..u2_8]) as [B, 8, M].

Strategy (8 NeuronCores, contraction sharding + per-step ReduceScatter):
  * core c owns the k-slice K_c = rows [128c,128c+128) of the M-dim.
    It holds column slices Ag[p][:, K_c] of every operator (as
    [k=128, m=1024] stationary tiles) and computes PARTIAL sums of
    every output row from its k-slice.
  * per step t, partials for all 8 m-tiles accumulate in PSUM as
    [re | im] pairs:  re += Ar@ur + (-Ai)@ui + Ar@x,
                      im += Ar@ui +   Ai@ur + Ai@x
    (-Ai is prepared host-side so no combine pass is needed).
  * the full PSUM [128, 4096] (all 1024 m-rows) is drained to DRAM and
    a ReduceScatter(add) gives core c the reduced rows K_c of u2_t --
    exactly the [ur|ui] rhs slice it needs for later steps.  RS output
    is 256KB vs the 2MB an AllGather would move, cutting the modeled
    collective cost per step from ~67us to ~21us.
  * x-only terms of step t+1 and recurrent terms using older u2_j are
    computed during the RS_t wait, so only the newest term
    (Ag[7] @ u2_t) sits on the serial chain.
  * step 8 computes/reduces only the real part.
  * everything runs in float32r (full-rate fp32 matmul mode on trn2).
"""

import numpy as np

B, L, M = 256, 8, 1024
N_CORES = 8
NT = M // 128  # 8 m-tiles of 128 rows
P_STEPS = 8

# Precision knobs (validated in sim and on HW; rel_fro ~6e-3 vs the 2e-2
# gate):
#   CC_BF16: feedback path (PSUM stage -> drain -> ReduceScatter -> slot)
#            and the final outputs in bf16, halving collective bytes.
#   W_BF16:  stationary A tiles AND x in bf16 (must match: neuronxcc
#            rejects mixed 32/16-bit matmul inputs), halving the load head.
CC_BF16 = True
W_BF16 = True

_CACHE = {}


def _build_program():
    import concourse.bacc as bacc
    import concourse.mybir as mybir
    import concourse.tile as tile
    from concourse.bass import ts

    dt = mybir.dt
    fr = dt.float32r
    f32 = dt.float32
    bf16 = dt.bfloat16
    wdt = bf16 if W_BF16 else fr
    cdt = bf16 if CC_BF16 else fr
    sdt = bf16 if CC_BF16 else f32  # stage / output element type

    nc = bacc.Bacc("TRN2", target_bir_lowering=False, debug=False,
                   num_devices=N_CORES)

    # Inputs (per core), partition-major layouts prepared on the host:
    #   wr/wi/wn: [k=128, p, m] -> [128, 8*1024]   (Ag[p][:,K_c] col slices;
    #             wn = -wi)
    #   xw:       [k=128, q, b] -> [128, 8*256]    (x k-slices)
    wr = nc.dram_tensor("wr", [128, L * M], wdt, kind="ExternalInput")
    wi = nc.dram_tensor("wi", [128, L * M], wdt, kind="ExternalInput")
    wn = nc.dram_tensor("wn", [128, L * M], wdt, kind="ExternalInput")
    # NB: real neuronxcc rejects mixed 32/16-bit matmul inputs, so xw must
    # match the weight dtype (CoreSim is laxer here).
    xw = nc.dram_tensor("xw", [128, L * 256], wdt, kind="ExternalInput")
    out = nc.dram_tensor("out", [P_STEPS - 1, 128, 256], sdt,
                         kind="ExternalOutput")
    # step 8 skips its ReduceScatter: each core emits its re partials for
    # all 1024 m-rows; the host sums the 8 cores' copies.
    out8 = nc.dram_tensor("out8", [M, 256], sdt, kind="ExternalOutput")

    # Collective buffers: cc_in holds this core's partial sums over all
    # 1024 m-rows; RS(add) leaves the reduced K_c shard in cc_out.
    cc_in = [
        nc.dram_tensor(f"cc_in{t}", [M, 512], cdt) for t in range(1, 8)
    ]
    cc_out = [
        nc.dram_tensor(f"cc_out{t}", [128, 512], cdt) for t in range(1, 8)
    ]

    rg = [list(range(N_CORES))]

    with tile.TileContext(nc) as tc:
        with (
            tc.tile_pool(name="a", bufs=1) as apool,
            tc.tile_pool(name="slot", bufs=7) as slpool,
            tc.tile_pool(name="stg", bufs=2) as stpool,
            tc.tile_pool(name="ps", bufs=1, space="PSUM") as pspool,
        ):
            t_wr = apool.tile([128, L * M], wdt, tag="wr")
            t_wi = apool.tile([128, L * M], wdt, tag="wi")
            t_wn = apool.tile([128, L * M], wdt, tag="wn")
            t_xw = apool.tile([128, L * 256], wdt, tag="xw")

            def wtile(which, pos, r):
                t = (t_wr, t_wi, t_wn)[which]
                return t[:, ts(pos * NT + r, 128)]

            slots = {}      # j -> sbuf tile [128, 512] = [ur | ui]

            # Zero weight tile for PE warm-up filler matmuls: the closing
            # matmuls of each step otherwise start from a cooled-down PE
            # (mid p-state, 2x cycle time) after idling during the RS wait.
            # Calibrated-count zero-contribution matmuls run back-to-back
            # into the close so it executes at full rate.
            t_wz = apool.tile([128, 128], wdt, tag="wz")
            nc.vector.memset(t_wz[:], 0.0)
            # per-step filler counts ~= measured pre-close PE idle (sim)
            WARMUP = {2: 113, 3: 83, 4: 76, 5: 60, 6: 25, 7: 18, 8: 108}

            # A/x loads: x first (small), then wr/wi per position so the
            # first matmuls don't wait on the whole 8MB.  wn is only
            # needed from step 2's close (after RS_1), so it is issued
            # after step 1's drain DMAs to keep them off the queue head.
            nc.sync.dma_start(t_xw[:], xw[:])
            for p in range(0, L, 2):
                sl = ts(p // 2, 2 * M)
                nc.sync.dma_start(t_wr[:, sl], wr[:, sl])
                nc.sync.dma_start(t_wi[:, sl], wi[:, sl])

            def emit_step(t):
                """Accumulate step t partials in PSUM, drain, ReduceScatter."""
                ps = pspool.tile([128, NT * 512], f32, tag="ps", name=f"ps{t}")
                started = [False] * NT

                def mm(r, half, which, pos, rhs, stop=False):
                    # half 0 = re region, 1 = im region of bank r
                    lo = r * 512 + half * 256
                    nc.tensor.matmul(
                        ps[:, lo:lo + 256], wtile(which, pos, r), rhs,
                        start=not started[r], stop=stop,
                        skip_group_check=True,
                    )
                    started[r] = True

                last = t == 8  # imaginary part never consumed
                # x-only terms: window position p carries x_{p+t-1}
                for p in range(0, 9 - t):
                    q = p + t - 1
                    rx = t_xw[:, ts(q, 256)]
                    x_stop = t == 1 and p == L - 1
                    for r in range(NT):
                        mm(r, 0, 0, p, rx, stop=x_stop)
                        if not last:
                            mm(r, 1, 1, p, rx, stop=x_stop)
                # recurrent terms, oldest first; j=t-1 closes the groups
                for d in range(WARMUP.get(t, 0)):
                    nc.tensor.matmul(
                        ps[:, 0:256], t_wz[:], t_xw[:, 0:256],
                        start=False, stop=False, skip_group_check=True,
                    )
                for j in range(1, t):
                    pos = 8 - t + j
                    ur = slots[j][:, 0:256]
                    ui = slots[j][:, 256:512]
                    cl = j == t - 1
                    for r in range(NT):
                        mm(r, 0, 0, pos, ur)
                        if not last:
                            mm(r, 1, 1, pos, ur)
                        mm(r, 0, 2, pos, ui, stop=cl)
                        if not last:
                            mm(r, 1, 0, pos, ui, stop=cl)

                # PSUM can't be DMA'd: stage bank r to SBUF as soon as its
                # groups close (DVE/Act alternating), drain per bank pair so
                # the DMA device pipelines behind the closing matmuls.
                # Step 8 drains its re partials straight to out8 (reduced on
                # the host instead of a final ReduceScatter).
                ci = out8 if last else cc_in[t - 1]
                w = 512 if not last else 256
                stg = stpool.tile([128, NT * w], sdt, tag="stg",
                                  name=f"stg{t}")
                for r in range(NT):
                    if last:
                        dst, src = stg[:, ts(r, 256)], ps[:, r * 512:r * 512 + 256]
                    else:
                        dst, src = stg[:, ts(r, 512)], ps[:, ts(r, 512)]
                    if r % 2 == 0:
                        nc.vector.tensor_copy(dst, src)
                    else:
                        nc.scalar.copy(dst, src)
                    if r % 2 == 1:
                        h = r // 2
                        nc.sync.dma_start(
                            ci[h * 256:(h + 1) * 256].rearrange(
                                "(r p) b -> p r b", r=2, p=128),
                            stg[:, (r - 1) * w:(r + 1) * w].rearrange(
                                "p (r b) -> p r b", r=2, b=w
                            ).bitcast(sdt if last else cdt),
                        )
                if last:
                    return
                co = cc_out[t - 1]
                nc.gpsimd.collective_compute(
                    "ReduceScatter", mybir.AluOpType.add,
                    replica_groups=rg, ins=[ci[:]], outs=[co[:]],
                )
                if t == 1:
                    # wn is first consumed by step 2's closing matmuls
                    # (during the RS_1 wait); issuing it after RS_1 keeps
                    # the 4MB off the DMA device ahead of step 1's drains.
                    for p in range(L):
                        sl = ts(p, M)
                        nc.sync.dma_start(t_wn[:, sl], wn[:, sl])
                # split ur/ui so the ur-only closing matmuls start ~1us
                # before the ui half lands
                slot = slpool.tile([128, 512], cdt, tag="slot",
                                   name=f"slot{t}")
                nc.sync.dma_start(slot[:, 0:256], co[:, 0:256])
                nc.sync.dma_start(slot[:, 256:512], co[:, 256:512])
                slots[t] = slot
                nc.sync.dma_start(out[t - 1], slot[:, 0:256].bitcast(sdt))

            for t in range(1, 9):
                emit_step(t)

    nc.compile()
    return nc


def _get_runner():
    if "runner" in _CACHE:
        return _CACHE["runner"]

    import jax
    from jax.sharding import Mesh, PartitionSpec
    from jax.experimental.shard_map import shard_map
    import concourse.mybir as mybir
    from concourse import bass2jax

    nc = _build_program()
    bass2jax.install_neuronx_cc_hook()
    partition_name = nc.partition_id_tensor.name if nc.partition_id_tensor else None
    in_names, out_names, out_avals, zero_outs = [], [], [], []
    for alloc in nc.m.functions[0].allocations:
        if not isinstance(alloc, mybir.MemoryLocationSet):
            continue
        name = alloc.memorylocations[0].name
        if alloc.kind == "ExternalInput":
            if name != partition_name:
                in_names.append(name)
        elif alloc.kind == "ExternalOutput":
            out_names.append(name)
            shape = tuple(alloc.tensor_shape)
            dtype = mybir.dt.np(alloc.dtype)
            out_avals.append(jax.core.ShapedArray(shape, dtype))
            zero_outs.append(np.zeros(shape, dtype))
    n_params = len(in_names)
    n_outs = len(out_avals)
    all_in = in_names + out_names + ([partition_name] if partition_name else [])
    donate = tuple(range(n_params, n_params + n_outs))

    def _body(*args):
        operands = list(args)
        if partition_name is not None:
            operands.append(bass2jax.partition_id_tensor())
        return tuple(
            bass2jax._bass_exec_p.bind(
                *operands,
                out_avals=tuple(out_avals),
                in_names=tuple(all_in),
                out_names=tuple(out_names),
                lowering_input_output_aliases=(),
                sim_require_finite=True,
                sim_require_nnan=True,
                nc=nc,
            )
        )

    devices = jax.devices()[:N_CORES]
    mesh = Mesh(np.asarray(devices), ("core",))
    sharded = jax.jit(
        shard_map(
            _body, mesh=mesh,
            in_specs=(PartitionSpec("core"),) * (n_params + n_outs),
            out_specs=(PartitionSpec("core"),) * n_outs,
            check_rep=False,
        ),
        donate_argnums=donate,
        keep_unused=True,
    )
    runner = {
        "sharded": sharded,
        "in_names": in_names,
        "out_names": out_names,
        "out_avals": out_avals,
        "zero_outs": zero_outs,
        "mesh": mesh,
    }
    _CACHE["runner"] = runner
    return runner


def prepare_inputs(x, A_real, A_imag):
    """Host-side reorder/transpose into the kernel's DMA-friendly layouts."""
    if W_BF16:
        import ml_dtypes
        wnp = np.dtype(ml_dtypes.bfloat16)
    else:
        wnp = np.dtype(np.float32)
    x = np.asarray(x, dtype=np.float32)
    A_real = np.asarray(A_real, dtype=np.float32)
    A_imag = np.asarray(A_imag, dtype=np.float32)
    idx = np.concatenate([[0], np.arange(L - 1, 0, -1)]).astype(np.int64)
    Agr = A_real[idx]  # [p, m, n(k)]
    Agi = A_imag[idx]
    # col slice transposed: [p, k in K_c, m] -> [128, p*m]
    AgrT = np.ascontiguousarray(Agr.transpose(0, 2, 1))  # [p, k, m]
    AgiT = np.ascontiguousarray(Agi.transpose(0, 2, 1))
    wrs, wis, wns = [], [], []
    for c in range(N_CORES):
        sl = AgrT[:, c * 128:(c + 1) * 128, :]  # [p, 128, m]
        w = np.ascontiguousarray(sl.transpose(1, 0, 2).reshape(128, -1))
        wrs.append(w.astype(wnp))
        sl = AgiT[:, c * 128:(c + 1) * 128, :]
        w = np.ascontiguousarray(sl.transpose(1, 0, 2).reshape(128, -1))
        wis.append(w.astype(wnp))
        wns.append(np.ascontiguousarray(-w).astype(wnp))
    # x: [b, q, m] -> per core [k=128, q, b]
    xws = []
    for c in range(N_CORES):
        xt = x[:, :, c * 128:(c + 1) * 128]  # [b, q, 128]
        xws.append(
            np.ascontiguousarray(xt.transpose(2, 1, 0).reshape(128, -1))
            .astype(wnp)
        )
    return wrs, wis, wns, xws


def kernel(x, A_real, A_imag, predict_length):
    P = int(predict_length)
    if P != P_STEPS:  # pragma: no cover - reference always uses 8
        return _numpy_fallback(x, A_real, A_imag, P)

    import jax

    runner = _get_runner()
    wrs, wis, wns, xws = prepare_inputs(x, A_real, A_imag)
    in_maps = [
        {"wr": wrs[c], "wi": wis[c], "wn": wns[c], "xw": xws[c]}
        for c in range(N_CORES)
    ]
    concat_in = [
        np.concatenate([m[n] for m in in_maps], axis=0) for n in runner["in_names"]
    ]
    czeros = [
        np.zeros((N_CORES * z.shape[0], *z.shape[1:]), z.dtype)
        for z in runner["zero_outs"]
    ]
    out_arrs = runner["sharded"](*concat_in, *czeros)
    jax.block_until_ready(out_arrs)
    o = np.asarray(out_arrs[0]).astype(np.float32)
    o = o.reshape(N_CORES, P_STEPS - 1, 128, 256)
    o8 = np.asarray(out_arrs[1]).astype(np.float32)
    o8 = o8.reshape(N_CORES, M, 256).sum(axis=0)  # reduce step-8 partials
    full = np.empty((B, P_STEPS, M), np.float32)
    # o: [c, t, p, b] -> [b, t, c*128+p]
    full[:, :P_STEPS - 1] = o.transpose(3, 1, 0, 2).reshape(B, P_STEPS - 1, M)
    full[:, P_STEPS - 1] = o8.T
    return np.ascontiguousarray(full)


def _numpy_fallback(x, A_real, A_imag, P):
    A = (np.asarray(A_real) + 1j * np.asarray(A_imag)).astype(np.complex64)
    idx = np.concatenate([[0], np.arange(L - 1, 0, -1)]).astype(np.int64)
    Ag = A[idx]
    uc = np.asarray(x).astype(np.complex64)
    for _ in range(P):
        u2 = np.einsum("kmn,bkn->bm", Ag, uc)
        uc = np.concatenate([uc[:, 1:], u2[:, None]], axis=1)
    return np.real(uc).astype(np.float32)



# revision 5
# speedup vs baseline: 1.7305x; 1.7305x over previous
"""Trainium2 Bass kernel for the 8-step complex DMD recurrence — v2.

Math (matching the reference):
  Ag[0]=A[0], Ag[p]=A[8-p] (p>=1), all complex [M,M].
  window w_t (len 8) starts as the real inputs x_0..x_7; each step
    u2_t = sum_p Ag[p] @ w_t[p]   (complex, [B,M])
  then the window slides.  Output = Re([u2_1..u2_8]) as [B, 8, M].

Strategy v2 (m-row sharding + per-step AllGather + 3-mult complex):
  * core c owns OUTPUT rows K_c = [128c, 128c+128) of the M dim and holds
    row slices Ag[p][K_c, :] of every operator.  Each step it computes the
    fully-reduced u2_t[K_c] locally (contraction over all of M) — no
    cross-core reduction needed, and each step's output rows stream
    straight to the per-core out tensor.
  * u2_t must then be broadcast for the next step's contraction: a 1-chip
    AllGather of the [128, 512] bf16 [re|im] shard (measured AG floor
    ~4.6us vs ReduceScatter ~7.3us; AG also avoids the CCE double-read).
  * complex products use the 3-multiplication (Gauss) form with
    host-precomputed stationaries War=Ar, Wd=Ai-Ar, Wn2=-(Ar+Ai):
      S  += Ar  @ (ur+ui)      (also re-part of x terms: S += Ar @ x)
      RE += Wn2 @ ui
      IM += Wd  @ ur           (also im-part of x terms: IM += Wd @ x)
      re = RE + S,  im = IM + S   (two DVE adds at drain time)
    -> 24 matmuls per complex window position instead of 32, and x terms
    need only War/Wd (Ai itself never ships).
  * per step the serial chain is just: close matmuls (newest u2 term) ->
    2 DVE adds -> DMA [128,512] -> AllGather -> slot DMA + us add.  All
    x-terms and older-u2 terms of the NEXT step are emitted between close
    and close, so the PE stays busy through the AllGather wait.
  * step 8 computes only re, accumulated in a single PSUM region.
"""

import numpy as np

B, L, M = 256, 8, 1024
N_CORES = 8
NK = M // 128   # 8 contraction tiles
P_STEPS = 8

IN_NAMES = ("war", "wd", "wn2", "xw")

_CACHE = {}


def _build_program():
    import concourse.bacc as bacc
    import concourse.mybir as mybir
    import concourse.tile as tile
    from concourse.bass import ts

    dt = mybir.dt
    f32 = dt.float32
    bf16 = dt.bfloat16
    wdt = bf16   # stationaries + x + slots
    cdt = bf16   # collective payload
    sdt = bf16   # stage / output element type

    nc = bacc.Bacc("TRN2", target_bir_lowering=False, debug=False,
                   num_devices=N_CORES)

    # Per-core inputs, partition-major layouts prepared on the host:
    #   war/wd: [k=128, (p,kt,m)] -> [128, 8*8*128]   row slices, transposed
    #   wn2:    [128, 7*8*128]                        (p>=1 only)
    #   xw:     [128, (q,kt,b)]  -> [128, 8*8*256]    full x, all cores
    war = nc.dram_tensor("war", [128, L * NK * 128], wdt, kind="ExternalInput")
    wd = nc.dram_tensor("wd", [128, L * NK * 128], wdt, kind="ExternalInput")
    wn2 = nc.dram_tensor("wn2", [128, (L - 1) * NK * 128], wdt,
                         kind="ExternalInput")
    xw = nc.dram_tensor("xw", [128, L * NK * 256], wdt, kind="ExternalInput")
    out = nc.dram_tensor("out", [P_STEPS, 128, 256], sdt,
                         kind="ExternalOutput")

    # Collective buffers (HBM): shard in, gathered u2_t out.
    cc_in = [nc.dram_tensor(f"cc_in{t}", [128, 512], cdt)
             for t in range(1, 8)]
    cc_out = [nc.dram_tensor(f"cc_out{t}", [M, 512], cdt)
              for t in range(1, 8)]

    rg = [list(range(N_CORES))]

    with tile.TileContext(nc) as tc:
        with (
            tc.tile_pool(name="a", bufs=1) as apool,
            tc.tile_pool(name="slot", bufs=7) as slpool,
            tc.tile_pool(name="stg", bufs=2) as stpool,
            tc.tile_pool(name="ps", bufs=3, space="PSUM") as pspool,
        ):
            t_war = apool.tile([128, L * NK * 128], wdt, tag="war")
            t_wd = apool.tile([128, L * NK * 128], wdt, tag="wd")
            t_wn2 = apool.tile([128, (L - 1) * NK * 128], wdt, tag="wn2")
            t_xw = apool.tile([128, L * NK * 256], wdt, tag="xw")

            def ar(p, k):
                return t_war[:, ts(p * NK + k, 128)]

            def wd_(p, k):
                return t_wd[:, ts(p * NK + k, 128)]

            def n2(p, k):
                return t_wn2[:, ts((p - 1) * NK + k, 128)]

            def xv(q, k):
                return t_xw[:, ts(q * NK + k, 256)]

            # Head loads: interleave x / war / wd per position so position-0
            # matmuls start after ~1MB instead of 8MB.  wn2 (first consumed
            # by step 2's close) queues after the step-1-critical loads.
            for p in range(L):
                nc.sync.dma_start(t_xw[:, ts(p, NK * 256)],
                                  xw[:, ts(p, NK * 256)])
                nc.sync.dma_start(t_war[:, ts(p, NK * 128)],
                                  war[:, ts(p, NK * 128)])
                nc.sync.dma_start(t_wd[:, ts(p, NK * 128)],
                                  wd[:, ts(p, NK * 128)])
            nc.sync.dma_start(t_wn2[:], wn2[:])

            slots = {}  # j -> sbuf tile [128, 8*768] = per-ktile [ur|ui|us]

            def slot_ap(j, k, part):
                lo = k * 768 + part * 256
                return slots[j][:, lo:lo + 256]

            # PSUM region bases within a [128, 1024] (= 2 bank) step tile.
            # PSUM group start/stop is BANK-granular (start zeroes the whole
            # 2KB bank), so S|RE share bank 0 with ONE start, IM is bank 1.
            S_, RE_, IM_ = 0, 256, 512

            def emit_step(t):
                last = t == 8
                ps = pspool.tile([128, 1024], f32, tag="ps", name=f"ps{t}")
                started = [False, False]  # per bank

                def mm(region, lhsT, rhs, stop=False):
                    bank = 0 if region < 512 else 1
                    nc.tensor.matmul(
                        ps[:, region:region + 256], lhsT, rhs,
                        start=not started[bank], stop=stop,
                        skip_group_check=True,
                    )
                    started[bank] = True

                # ---- phase A: x terms + older-u2 terms (no newest dep) ----
                if not last:
                    for p in range(0, 9 - t):
                        q = p + t - 1
                        # t=1 has no close phase: its x groups carry the stops
                        cl = t == 1 and p == 8 - t
                        for k in range(NK):
                            st = cl and k == NK - 1
                            mm(S_, ar(p, k), xv(q, k), stop=st)
                            mm(IM_, wd_(p, k), xv(q, k), stop=st)
                    for j in range(1, t - 1):
                        pos = 8 - t + j
                        for k in range(NK):
                            mm(S_, ar(pos, k), slot_ap(j, k, 2))
                            mm(RE_, n2(pos, k), slot_ap(j, k, 1))
                            mm(IM_, wd_(pos, k), slot_ap(j, k, 0))
                else:
                    # t=8: re only, all into the RE region. pos = j here.
                    for k in range(NK):
                        mm(RE_, ar(0, k), xv(7, k))
                    for j in range(1, 7):
                        for k in range(NK):
                            mm(RE_, ar(j, k), slot_ap(j, k, 2))
                            mm(RE_, n2(j, k), slot_ap(j, k, 1))

                # ---- close: newest term j = t-1 (for t >= 2) ----
                if t >= 2:
                    j = t - 1
                    pos = 7 if not last else 7
                    for k in range(NK):
                        if not last:
                            mm(RE_, n2(pos, k), slot_ap(j, k, 1),
                               stop=k == NK - 1)
                            mm(IM_, wd_(pos, k), slot_ap(j, k, 0),
                               stop=k == NK - 1)
                            mm(S_, ar(pos, k), slot_ap(j, k, 2),
                               stop=k == NK - 1)
                        else:
                            mm(RE_, n2(pos, k), slot_ap(j, k, 1))
                            mm(RE_, ar(pos, k), slot_ap(j, k, 2),
                               stop=k == NK - 1)
                # ---- drain / combine ----
                if last:
                    stg = stpool.tile([128, 256], sdt, tag="stg8")
                    nc.vector.tensor_copy(stg[:], ps[:, RE_:RE_ + 256])
                    nc.sync.dma_start(out[7], stg[:])
                    return
                # DVE tensor_tensor cannot read two PSUM operands (neuronxcc
                # verifier); stage S to SBUF f32 on the ACT engine first.
                stg = stpool.tile([128, 512], sdt, tag="stg", name=f"stg{t}")
                s_f32 = stpool.tile([128, 256], f32, tag="sf32",
                                    name=f"sf32_{t}")
                nc.scalar.copy(s_f32[:], ps[:, S_:S_ + 256])
                if t == 1:
                    nc.vector.tensor_copy(stg[:, 0:256], ps[:, S_:S_ + 256])
                else:
                    nc.vector.tensor_add(stg[:, 0:256], ps[:, RE_:RE_ + 256],
                                         s_f32[:])
                nc.vector.tensor_add(stg[:, 256:512], ps[:, IM_:IM_ + 256],
                                     s_f32[:])
                nc.sync.dma_start(out[t - 1], stg[:, 0:256])
                nc.sync.dma_start(cc_in[t - 1][:], stg[:])
                nc.gpsimd.collective_compute(
                    "AllGather", mybir.AluOpType.bypass,
                    replica_groups=rg,
                    ins=[cc_in[t - 1][:]], outs=[cc_out[t - 1][:]],
                )
                slot = slpool.tile([128, NK * 768], wdt, tag="slot",
                                   name=f"slot{t}")
                for k in range(NK):
                    nc.sync.dma_start(
                        slot[:, k * 768:k * 768 + 512],
                        cc_out[t - 1][k * 128:(k + 1) * 128, :],
                    )
                    nc.vector.tensor_add(
                        slot[:, k * 768 + 512:k * 768 + 768],
                        slot[:, k * 768:k * 768 + 256],
                        slot[:, k * 768 + 256:k * 768 + 512],
                    )
                slots[t] = slot

            for t in range(1, 9):
                emit_step(t)

    nc.compile()
    return nc


def _get_runner():
    if "runner" in _CACHE:
        return _CACHE["runner"]

    import jax
    from jax.sharding import Mesh, PartitionSpec
    from jax.experimental.shard_map import shard_map
    import concourse.mybir as mybir
    from concourse import bass2jax

    nc = _build_program()
    bass2jax.install_neuronx_cc_hook()
    partition_name = nc.partition_id_tensor.name if nc.partition_id_tensor else None
    in_names, out_names, out_avals, zero_outs = [], [], [], []
    for alloc in nc.m.functions[0].allocations:
        if not isinstance(alloc, mybir.MemoryLocationSet):
            continue
        name = alloc.memorylocations[0].name
        if alloc.kind == "ExternalInput":
            if name != partition_name:
                in_names.append(name)
        elif alloc.kind == "ExternalOutput":
            out_names.append(name)
            shape = tuple(alloc.tensor_shape)
            dtype = mybir.dt.np(alloc.dtype)
            out_avals.append(jax.core.ShapedArray(shape, dtype))
            zero_outs.append(np.zeros(shape, dtype))
    n_params = len(in_names)
    n_outs = len(out_avals)
    all_in = in_names + out_names + ([partition_name] if partition_name else [])
    donate = tuple(range(n_params, n_params + n_outs))

    def _body(*args):
        operands = list(args)
        if partition_name is not None:
            operands.append(bass2jax.partition_id_tensor())
        return tuple(
            bass2jax._bass_exec_p.bind(
                *operands,
                out_avals=tuple(out_avals),
                in_names=tuple(all_in),
                out_names=tuple(out_names),
                lowering_input_output_aliases=(),
                sim_require_finite=True,
                sim_require_nnan=True,
                nc=nc,
            )
        )

    devices = jax.devices()[:N_CORES]
    mesh = Mesh(np.asarray(devices), ("core",))
    sharded = jax.jit(
        shard_map(
            _body, mesh=mesh,
            in_specs=(PartitionSpec("core"),) * (n_params + n_outs),
            out_specs=(PartitionSpec("core"),) * n_outs,
            check_rep=False,
        ),
        donate_argnums=donate,
        keep_unused=True,
    )
    runner = {
        "sharded": sharded,
        "in_names": in_names,
        "out_names": out_names,
        "out_avals": out_avals,
        "zero_outs": zero_outs,
        "mesh": mesh,
    }
    _CACHE["runner"] = runner
    return runner


def prepare_inputs(x, A_real, A_imag):
    """Host-side reorder/transpose into the kernel's DMA-friendly layouts."""
    import ml_dtypes
    wnp = np.dtype(ml_dtypes.bfloat16)
    x = np.asarray(x, dtype=np.float32)
    A_real = np.asarray(A_real, dtype=np.float32)
    A_imag = np.asarray(A_imag, dtype=np.float32)
    idx = np.concatenate([[0], np.arange(L - 1, 0, -1)]).astype(np.int64)
    Agr = A_real[idx]          # [p, m, n(k)]
    Agi = A_imag[idx]
    D = Agi - Agr
    N2 = -(Agr + Agi)
    # transposed views [p, k, m]
    AgrT = np.ascontiguousarray(Agr.transpose(0, 2, 1)).astype(wnp)
    DT = np.ascontiguousarray(D.transpose(0, 2, 1)).astype(wnp)
    N2T = np.ascontiguousarray(N2.transpose(0, 2, 1)).astype(wnp)

    def percore(matT, c, p_lo):
        # matT: [p, k(M), m(M)] bf16 -> [128, (p,kt,m)] for rows K_c
        sl = matT[p_lo:, :, c * 128:(c + 1) * 128]    # [P, 1024, 128]
        P = sl.shape[0]
        sl = sl.reshape(P, NK, 128, 128)               # [P, kt, kk, m]
        sl = sl.transpose(2, 0, 1, 3)                  # [kk, P, kt, m]
        return np.ascontiguousarray(sl.reshape(128, P * NK * 128))

    wars = [percore(AgrT, c, 0) for c in range(N_CORES)]
    wds = [percore(DT, c, 0) for c in range(N_CORES)]
    wn2s = [percore(N2T, c, 1) for c in range(N_CORES)]
    # x: [b, q, m] -> [128, (q, kt, b)], identical on every core
    xt = x.transpose(2, 1, 0).astype(wnp)              # [M, q, B]
    xt = xt.reshape(NK, 128, L, B).transpose(1, 2, 0, 3)  # [kk, q, kt, b]
    xw = np.ascontiguousarray(xt.reshape(128, L * NK * B))
    xws = [xw] * N_CORES
    return wars, wds, wn2s, xws


def kernel(x, A_real, A_imag, predict_length):
    P = int(predict_length)
    if P != P_STEPS:  # pragma: no cover - reference always uses 8
        return _numpy_fallback(x, A_real, A_imag, P)

    import jax

    runner = _get_runner()
    wars, wds, wn2s, xws = prepare_inputs(x, A_real, A_imag)
    in_maps = [
        {"war": wars[c], "wd": wds[c], "wn2": wn2s[c], "xw": xws[c]}
        for c in range(N_CORES)
    ]
    concat_in = [
        np.concatenate([m[n] for m in in_maps], axis=0)
        for n in runner["in_names"]
    ]
    czeros = [
        np.zeros((N_CORES * z.shape[0], *z.shape[1:]), z.dtype)
        for z in runner["zero_outs"]
    ]
    out_arrs = runner["sharded"](*concat_in, *czeros)
    jax.block_until_ready(out_arrs)
    o = np.asarray(out_arrs[0]).astype(np.float32)
    o = o.reshape(N_CORES, P_STEPS, 128, 256)
    # o: [c, t, r, b] -> [b, t, c*128+r]
    full = np.ascontiguousarray(
        o.transpose(3, 1, 0, 2).reshape(B, P_STEPS, M))
    return full


def _numpy_fallback(x, A_real, A_imag, P):
    A = (np.asarray(A_real) + 1j * np.asarray(A_imag)).astype(np.complex64)
    idx = np.concatenate([[0], np.arange(L - 1, 0, -1)]).astype(np.int64)
    Ag = A[idx]
    uc = np.asarray(x).astype(np.complex64)
    for _ in range(P):
        u2 = np.einsum("kmn,bkn->bm", Ag, uc)
        uc = np.concatenate([uc[:, 1:], u2[:, None]], axis=1)
    return np.real(uc).astype(np.float32)


# revision 6
# speedup vs baseline: 1.7567x; 1.0151x over previous
"""Trainium2 Bass kernel for the 8-step complex DMD recurrence — v2.

Math (matching the reference):
  Ag[0]=A[0], Ag[p]=A[8-p] (p>=1), all complex [M,M].
  window w_t (len 8) starts as the real inputs x_0..x_7; each step
    u2_t = sum_p Ag[p] @ w_t[p]   (complex, [B,M])
  then the window slides.  Output = Re([u2_1..u2_8]) as [B, 8, M].

Strategy v2 (m-row sharding + per-step AllGather + 3-mult complex):
  * core c owns OUTPUT rows K_c = [128c, 128c+128) of the M dim and holds
    row slices Ag[p][K_c, :] of every operator.  Each step it computes the
    fully-reduced u2_t[K_c] locally (contraction over all of M) — no
    cross-core reduction needed, and each step's output rows stream
    straight to the per-core out tensor.
  * u2_t must then be broadcast for the next step's contraction: a 1-chip
    AllGather of the [128, 512] bf16 [re|im] shard (measured AG floor
    ~4.6us vs ReduceScatter ~7.3us; AG also avoids the CCE double-read).
  * complex products use the 3-multiplication (Gauss) form with
    host-precomputed stationaries War=Ar, Wd=Ai-Ar, Wn2=-(Ar+Ai):
      S  += Ar  @ (ur+ui)      (also re-part of x terms: S += Ar @ x)
      RE += Wn2 @ ui
      IM += Wd  @ ur           (also im-part of x terms: IM += Wd @ x)
      re = RE + S,  im = IM + S   (two DVE adds at drain time)
    -> 24 matmuls per complex window position instead of 32, and x terms
    need only War/Wd (Ai itself never ships).
  * per step the serial chain is just: close matmuls (newest u2 term) ->
    ACT stage + 2 DVE adds -> DMA [128,512] -> AllGather -> slot DMA +
    us add.  The x-terms and older-u2 terms of each step are emitted
    before its close, keeping the PE busy through the AllGather wait
    (deeper lookahead measured slower: it delays the close on the
    in-order PE queue).
  * step 8 computes only re, accumulated in a single PSUM region.
"""

import numpy as np

B, L, M = 256, 8, 1024
N_CORES = 8
NK = M // 128   # 8 contraction tiles
P_STEPS = 8

IN_NAMES = ("war", "wd", "wn2", "xw")

_CACHE = {}


def _build_program(reps=1, variant="full", pipeline=False):
    import concourse.bacc as bacc
    import concourse.mybir as mybir
    import concourse.tile as tile
    from concourse.bass import ts

    dt = mybir.dt
    f32 = dt.float32
    bf16 = dt.bfloat16
    wdt = bf16   # stationaries + x + slots
    cdt = bf16   # collective payload
    sdt = bf16   # stage / output element type

    nc = bacc.Bacc("TRN2", target_bir_lowering=False, debug=False,
                   num_devices=N_CORES)

    # Per-core inputs, partition-major layouts prepared on the host:
    #   war/wd: [k=128, (p,kt,m)] -> [128, 8*8*128]   row slices, transposed
    #   wn2:    [128, 7*8*128]                        (p>=1 only)
    #   xw:     [128, (q,kt,b)]  -> [128, 8*8*256]    full x, all cores
    war = nc.dram_tensor("war", [128, L * NK * 128], wdt, kind="ExternalInput")
    wd = nc.dram_tensor("wd", [128, L * NK * 128], wdt, kind="ExternalInput")
    wn2 = nc.dram_tensor("wn2", [128, (L - 1) * NK * 128], wdt,
                         kind="ExternalInput")
    xw = nc.dram_tensor("xw", [128, L * NK * 256], wdt, kind="ExternalInput")
    out = nc.dram_tensor("out", [P_STEPS, 128, 256], sdt,
                         kind="ExternalOutput")

    # Collective buffers (HBM): shard in, gathered u2_t out. One set per
    # rep so benchmark replication adds no false cross-rep dependencies.
    cc_in = [[nc.dram_tensor(f"cc_in{r}_{t}", [128, 512], cdt)
              for t in range(1, 8)] for r in range(reps)]
    cc_out = [[nc.dram_tensor(f"cc_out{r}_{t}", [M, 512], cdt)
               for t in range(1, 8)] for r in range(reps)]

    rg = [list(range(N_CORES))]

    with tile.TileContext(nc) as tc:
        with (
            tc.tile_pool(name="a", bufs=1) as apool,
            tc.tile_pool(name="slot", bufs=7) as slpool,
            tc.tile_pool(name="stg", bufs=2) as stpool,
            tc.tile_pool(name="ps", bufs=4, space="PSUM") as pspool,
        ):
            t_war = apool.tile([128, L * NK * 128], wdt, tag="war")
            t_wd = apool.tile([128, L * NK * 128], wdt, tag="wd")
            t_wn2 = apool.tile([128, (L - 1) * NK * 128], wdt, tag="wn2")
            t_xw = apool.tile([128, L * NK * 256], wdt, tag="xw")

            def ar(p, k):
                return t_war[:, ts(p * NK + k, 128)]

            def wd_(p, k):
                return t_wd[:, ts(p * NK + k, 128)]

            def n2(p, k):
                return t_wn2[:, ts((p - 1) * NK + k, 128)]

            def xv(q, k):
                return t_xw[:, ts(q * NK + k, 256)]

            # Head loads: interleave x / war / wd per position so position-0
            # matmuls start after ~1MB instead of 8MB.  wn2 (first consumed
            # by step 2's close, which always uses position 7) queues after
            # the step-1-critical loads, position 7 first.
            for p in range(L):
                nc.sync.dma_start(t_xw[:, ts(p, NK * 256)],
                                  xw[:, ts(p, NK * 256)])
                nc.sync.dma_start(t_war[:, ts(p, NK * 128)],
                                  war[:, ts(p, NK * 128)])
                nc.sync.dma_start(t_wd[:, ts(p, NK * 128)],
                                  wd[:, ts(p, NK * 128)])
            for p in (7, 1, 2, 3, 4, 5, 6):
                nc.sync.dma_start(t_wn2[:, ts(p - 1, NK * 128)],
                                  wn2[:, ts(p - 1, NK * 128)])

            slots = {}  # (rep, j) -> sbuf tile [128, 8*768] per-ktile [ur|ui|us]
            if variant == "nocc":
                dummy = slpool.tile([128, NK * 768], wdt, tag="dummy")
                nc.vector.memset(dummy[:], 0.0)
                slots["dummy"] = dummy

            rep = 0

            def slot_ap(j, k, part):
                lo = k * 768 + part * 256
                return slots[rep, j][:, lo:lo + 256]

            # PSUM region bases within a [128, 1024] (= 2 bank) step tile.
            # PSUM group start/stop is BANK-granular (start zeroes the whole
            # 2KB bank), so S|RE share bank 0 with ONE start, IM is bank 1.
            S_, RE_, IM_ = 0, 256, 512

            pss = {}      # (rep, t) -> psum tile
            started = {}  # (rep, t) -> [bank0, bank1]

            def mm(t, region, lhsT, rhs, stop=False):
                st = started[rep, t]
                bank = 0 if region < 512 else 1
                nc.tensor.matmul(
                    pss[rep, t][:, region:region + 256], lhsT, rhs,
                    start=not st[bank], stop=stop,
                    skip_group_check=True,
                )
                st[bank] = True

            def emit_xA(t):
                """x-only terms of step t — no u2 dependencies at all."""
                pss[rep, t] = pspool.tile([128, 1024], f32, tag="ps",
                                          name=f"ps{rep}_{t}")
                started[rep, t] = [False, False]
                if t < 8:
                    for p in range(0, 9 - t):
                        q = p + t - 1
                        # t=1 has no close phase: its x groups carry the stops
                        cl = t == 1 and p == 8 - t
                        for k in range(NK):
                            st = cl and k == NK - 1
                            mm(t, S_, ar(p, k), xv(q, k), stop=st)
                            mm(t, IM_, wd_(p, k), xv(q, k), stop=st)
                else:
                    for k in range(NK):
                        mm(t, RE_, ar(0, k), xv(7, k))

            def emit_uA(t):
                """older-u2 terms of step t (slots j <= t-2)."""
                for j in range(1, t - 1):
                    pos = 8 - t + j
                    for k in range(NK):
                        if t < 8:
                            mm(t, S_, ar(pos, k), slot_ap(j, k, 2))
                            mm(t, RE_, n2(pos, k), slot_ap(j, k, 1))
                            mm(t, IM_, wd_(pos, k), slot_ap(j, k, 0))
                        else:
                            mm(t, RE_, ar(j, k), slot_ap(j, k, 2))
                            mm(t, RE_, n2(j, k), slot_ap(j, k, 1))

            def emit_close(t):
                """newest term j = t-1, position 7 — waits on slot t-1."""
                j = t - 1
                for k in range(NK):
                    if t < 8:
                        mm(t, RE_, n2(7, k), slot_ap(j, k, 1),
                           stop=k == NK - 1)
                        mm(t, IM_, wd_(7, k), slot_ap(j, k, 0),
                           stop=k == NK - 1)
                        mm(t, S_, ar(7, k), slot_ap(j, k, 2),
                           stop=k == NK - 1)
                    else:
                        mm(t, RE_, n2(7, k), slot_ap(j, k, 1))
                        mm(t, RE_, ar(7, k), slot_ap(j, k, 2),
                           stop=k == NK - 1)

            def emit_epilogue(t):
                last = t == 8
                ps = pss[rep, t]
                if last:
                    stg = stpool.tile([128, 256], sdt, tag="stg8", name=f"stg8_{rep}")
                    nc.vector.tensor_copy(stg[:], ps[:, RE_:RE_ + 256])
                    nc.sync.dma_start(out[7], stg[:])
                    return
                # DVE tensor_tensor cannot read two PSUM operands (neuronxcc
                # verifier); stage S to SBUF f32 on the ACT engine first.
                stg = stpool.tile([128, 512], sdt, tag="stg", name=f"stg{rep}_{t}")
                s_f32 = stpool.tile([128, 256], f32, tag="sf32",
                                    name=f"sf32_{rep}_{t}")
                nc.scalar.copy(s_f32[:], ps[:, S_:S_ + 256])
                if t == 1:
                    nc.vector.tensor_copy(stg[:, 0:256], ps[:, S_:S_ + 256])
                else:
                    nc.vector.tensor_add(stg[:, 0:256], ps[:, RE_:RE_ + 256],
                                         s_f32[:])
                nc.vector.tensor_add(stg[:, 256:512], ps[:, IM_:IM_ + 256],
                                     s_f32[:])
                # out DMA rides the ACT engine's HWDGE ring so it never
                # queues ahead of the chain-critical cc_in DMA on SP's ring.
                nc.scalar.dma_start(out[t - 1], stg[:, 0:256])
                if variant == "nocc":
                    slots[rep, t] = slots["dummy"]
                    return
                nc.sync.dma_start(cc_in[rep][t - 1][:], stg[:])
                if variant == "full":
                    nc.gpsimd.collective_compute(
                        "AllGather", mybir.AluOpType.bypass,
                        replica_groups=rg,
                        ins=[cc_in[rep][t - 1][:]],
                        outs=[cc_out[rep][t - 1][:]],
                    )
                slot = slpool.tile([128, NK * 768], wdt, tag="slot",
                                   name=f"slot{rep}_{t}")
                for k in range(NK):
                    if variant == "full":
                        src_ap = cc_out[rep][t - 1][k * 128:(k + 1) * 128, :]
                    else:
                        src_ap = cc_in[rep][t - 1][:]
                    nc.sync.dma_start(
                        slot[:, k * 768:k * 768 + 512], src_ap)
                    nc.vector.tensor_add(
                        slot[:, k * 768 + 512:k * 768 + 768],
                        slot[:, k * 768:k * 768 + 256],
                        slot[:, k * 768 + 256:k * 768 + 512],
                    )
                slots[rep, t] = slot

            # Software-pipelined emission: before each close(t) the PE queue
            # holds xA(t+1) + uA(t) — enough independent work to cover the
            # drain->DMA->AllGather->slot-DMA serial chain of step t-1.
            for rep in range(reps):
                if pipeline:
                    emit_xA(1)
                    emit_epilogue(1)
                    emit_xA(2)
                    for t in range(2, 9):
                        if t < 8:
                            emit_xA(t + 1)
                        emit_uA(t)
                        emit_close(t)
                        emit_epilogue(t)
                else:
                    for t in range(1, 9):
                        emit_xA(t)
                        emit_uA(t)
                        if t >= 2:
                            emit_close(t)
                        emit_epilogue(t)

    nc.compile()
    return nc


def _get_runner():
    if "runner" in _CACHE:
        return _CACHE["runner"]

    import jax
    from jax.sharding import Mesh, PartitionSpec
    from jax.experimental.shard_map import shard_map
    import concourse.mybir as mybir
    from concourse import bass2jax

    nc = _build_program()
    bass2jax.install_neuronx_cc_hook()
    partition_name = nc.partition_id_tensor.name if nc.partition_id_tensor else None
    in_names, out_names, out_avals, zero_outs = [], [], [], []
    for alloc in nc.m.functions[0].allocations:
        if not isinstance(alloc, mybir.MemoryLocationSet):
            continue
        name = alloc.memorylocations[0].name
        if alloc.kind == "ExternalInput":
            if name != partition_name:
                in_names.append(name)
        elif alloc.kind == "ExternalOutput":
            out_names.append(name)
            shape = tuple(alloc.tensor_shape)
            dtype = mybir.dt.np(alloc.dtype)
            out_avals.append(jax.core.ShapedArray(shape, dtype))
            zero_outs.append(np.zeros(shape, dtype))
    n_params = len(in_names)
    n_outs = len(out_avals)
    all_in = in_names + out_names + ([partition_name] if partition_name else [])
    donate = tuple(range(n_params, n_params + n_outs))

    def _body(*args):
        operands = list(args)
        if partition_name is not None:
            operands.append(bass2jax.partition_id_tensor())
        return tuple(
            bass2jax._bass_exec_p.bind(
                *operands,
                out_avals=tuple(out_avals),
                in_names=tuple(all_in),
                out_names=tuple(out_names),
                lowering_input_output_aliases=(),
                sim_require_finite=True,
                sim_require_nnan=True,
                nc=nc,
            )
        )

    devices = jax.devices()[:N_CORES]
    mesh = Mesh(np.asarray(devices), ("core",))
    sharded = jax.jit(
        shard_map(
            _body, mesh=mesh,
            in_specs=(PartitionSpec("core"),) * (n_params + n_outs),
            out_specs=(PartitionSpec("core"),) * n_outs,
            check_rep=False,
        ),
        donate_argnums=donate,
        keep_unused=True,
    )
    runner = {
        "sharded": sharded,
        "in_names": in_names,
        "out_names": out_names,
        "out_avals": out_avals,
        "zero_outs": zero_outs,
        "mesh": mesh,
    }
    _CACHE["runner"] = runner
    return runner


def prepare_inputs(x, A_real, A_imag):
    """Host-side reorder/transpose into the kernel's DMA-friendly layouts."""
    import ml_dtypes
    wnp = np.dtype(ml_dtypes.bfloat16)
    x = np.asarray(x, dtype=np.float32)
    A_real = np.asarray(A_real, dtype=np.float32)
    A_imag = np.asarray(A_imag, dtype=np.float32)
    idx = np.concatenate([[0], np.arange(L - 1, 0, -1)]).astype(np.int64)
    Agr = A_real[idx]          # [p, m, n(k)]
    Agi = A_imag[idx]
    D = Agi - Agr
    N2 = -(Agr + Agi)
    # transposed views [p, k, m]
    AgrT = np.ascontiguousarray(Agr.transpose(0, 2, 1)).astype(wnp)
    DT = np.ascontiguousarray(D.transpose(0, 2, 1)).astype(wnp)
    N2T = np.ascontiguousarray(N2.transpose(0, 2, 1)).astype(wnp)

    def percore(matT, c, p_lo):
        # matT: [p, k(M), m(M)] bf16 -> [128, (p,kt,m)] for rows K_c
        sl = matT[p_lo:, :, c * 128:(c + 1) * 128]    # [P, 1024, 128]
        P = sl.shape[0]
        sl = sl.reshape(P, NK, 128, 128)               # [P, kt, kk, m]
        sl = sl.transpose(2, 0, 1, 3)                  # [kk, P, kt, m]
        return np.ascontiguousarray(sl.reshape(128, P * NK * 128))

    wars = [percore(AgrT, c, 0) for c in range(N_CORES)]
    wds = [percore(DT, c, 0) for c in range(N_CORES)]
    wn2s = [percore(N2T, c, 1) for c in range(N_CORES)]
    # x: [b, q, m] -> [128, (q, kt, b)], identical on every core
    xt = x.transpose(2, 1, 0).astype(wnp)              # [M, q, B]
    xt = xt.reshape(NK, 128, L, B).transpose(1, 2, 0, 3)  # [kk, q, kt, b]
    xw = np.ascontiguousarray(xt.reshape(128, L * NK * B))
    xws = [xw] * N_CORES
    return wars, wds, wn2s, xws


def kernel(x, A_real, A_imag, predict_length):
    P = int(predict_length)
    if P != P_STEPS:  # pragma: no cover - reference always uses 8
        return _numpy_fallback(x, A_real, A_imag, P)

    import jax

    runner = _get_runner()
    wars, wds, wn2s, xws = prepare_inputs(x, A_real, A_imag)
    in_maps = [
        {"war": wars[c], "wd": wds[c], "wn2": wn2s[c], "xw": xws[c]}
        for c in range(N_CORES)
    ]
    concat_in = [
        np.concatenate([m[n] for m in in_maps], axis=0)
        for n in runner["in_names"]
    ]
    czeros = [
        np.zeros((N_CORES * z.shape[0], *z.shape[1:]), z.dtype)
        for z in runner["zero_outs"]
    ]
    out_arrs = runner["sharded"](*concat_in, *czeros)
    jax.block_until_ready(out_arrs)
    o = np.asarray(out_arrs[0]).astype(np.float32)
    o = o.reshape(N_CORES, P_STEPS, 128, 256)
    # o: [c, t, r, b] -> [b, t, c*128+r]
    full = np.ascontiguousarray(
        o.transpose(3, 1, 0, 2).reshape(B, P_STEPS, M))
    return full


def _numpy_fallback(x, A_real, A_imag, P):
    A = (np.asarray(A_real) + 1j * np.asarray(A_imag)).astype(np.complex64)
    idx = np.concatenate([[0], np.arange(L - 1, 0, -1)]).astype(np.int64)
    Ag = A[idx]
    uc = np.asarray(x).astype(np.complex64)
    for _ in range(P):
        u2 = np.einsum("kmn,bkn->bm", Ag, uc)
        uc = np.concatenate([uc[:, 1:], u2[:, None]], axis=1)
    return np.real(uc).astype(np.float32)
